# revision 35
# baseline (speedup 1.0000x reference)
"""Trainium2 Bass kernel for nn_AttentionHead_5583457485447 (sparse_attention).

Reference computation (per batch b):
    q = X @ Wq; k = X @ Wk                      # [N, DK]
    s = relu((q @ k.T) / sqrt(DK)) * M_mask     # [N, N]
    out = s @ Z @ Wv                            # [N, DV]

Strategy (8 NeuronCores, data-parallel over batch B=8, one batch per core):
  - Fold 1/sqrt(DK) into Wv (relu is positively homogeneous, rest is linear).
  - Fold Wv into Z on device: ZW = Z @ (Wv/8), so the N x N masked-score
    matrix feeds ONE big matmul: out = masked @ ZW.
  - Compute the score matrix directly in transposed [m, n] layout
    (lhsT = kT, rhs = qT), so it is already in the lhsT layout the second
    matmul needs (contraction over m => m on partitions). No on-chip
    transposes of the N x N matrix.
  - Mask is fed pre-transposed from the host (layout prep only).
  - relu + mask: rotated across engines -- fused DVE scalar_tensor_tensor
    (max(s,0)*maskT), or ACT relu followed by a bf16 DVE/GpSimd multiply.
  - All matmul inputs are bf16 (host-cast); every accumulation is fp32 in
    PSUM. Score matmuls row-pack two K=64 m-chunks into one PE pass via
    tile_position. Two n-half passes let C(half0) overlap B(half1) on the
    PE; a dummy-matmul warm-up engages the HAM full clock early.
"""

import json
import os
import sys

import numpy as np

B, N, D, DK = 8, 2048, 256, 64
DV = D + 1  # 257
NT = N // 128  # 16 tiles along n and along m
PW = 512  # scores matmul moving width
SW = 1024  # fused relu-mask op width (psum tile width, 2 banks)

LAST_EXEC_NS = None
_CACHE = {}


# --------------------------------------------------------------------------
# Patch 1: this container's walrus build rejects instructions carrying more
# than one semaphore wait. Split excess waits onto same-engine NOPs at the
# serialized-BIR level (generic, covers Tile's drains and compute ops).
# --------------------------------------------------------------------------
def _split_waits_in_bir(bir_json: bytes) -> bytes:
    bir = json.loads(bir_json)
    changed = False
    drop_ldw = os.environ.get("KERNEL_DROP_LDW", "0") == "1"
    for fn in bir.get("functions", []):
        for bb in fn.get("blocks", []):
            insts = bb.get("instructions", [])
            if drop_ldw:
                # Remove standalone Ldweights prefetches (the paired Matmult is
                # self-loading: it carries both operands). Merge their sync
                # info into the following Matmult on the same engine.
                merged = []
                pend = {}
                for inst in insts:
                    if inst.get("opcode") == "Ldweights":
                        si = inst.get("sync_info") or {}
                        if si.get("on_wait") or si.get("on_update"):
                            pend.setdefault(inst["engine"], []).append(si)
                        changed = True
                        continue
                    if inst.get("opcode") == "Matmult" and pend.get(inst.get("engine")):
                        tgt = inst.setdefault("sync_info", {"on_update": [], "on_wait": []})
                        tgt.setdefault("on_wait", [])
                        tgt.setdefault("on_update", [])
                        for si in pend.pop(inst["engine"]):
                            tgt["on_wait"] += si.get("on_wait") or []
                            tgt["on_update"] += si.get("on_update") or []
                    merged.append(inst)
                insts = merged
            out = []
            for inst in insts:
                si = inst.get("sync_info")
                ow = (si or {}).get("on_wait") or []
                if len(ow) > 1:
                    changed = True
                    for i, w in enumerate(ow[:-1]):
                        out.append({
                            "debug": inst.get("debug", 0),
                            "engine": inst["engine"],
                            "ins": [],
                            "name": f"{inst['name']}-ws{i}",
                            "opcode": "NoOp",
                            "outs": [],
                            "sync_info": {"on_update": [], "on_wait": [w]},
                            "text_hint": "wait_split",
                        })
                    si["on_wait"] = [ow[-1]]
                out.append(inst)
            bb["instructions"] = out
    return json.dumps(bir).encode() if changed else bir_json


def _apply_bir_patch():
    import concourse.bass_utils as bass_utils
    import concourse.bass2jax as bass2jax

    if os.environ.get("KERNEL_LDW_OPT", "0") == "1":
        rc_orig = bass_utils.run_command
        if not getattr(rc_orig, "_ldw_wrapped", False):
            def rc_wrapped(argv, **kwargs):
                argv = [a.replace("--enable-ldw-opt=false", "--enable-ldw-opt=true")
                        if isinstance(a, str) else a for a in argv]
                return rc_orig(argv, **kwargs)
            rc_wrapped._ldw_wrapped = True
            bass_utils.run_command = rc_wrapped

    orig = bass_utils.compile_bir_kernel
    if getattr(orig, "_wait_split_wrapped", False):
        return

    def wrapped(bir_json, tmpdir, neff_name="file.neff"):
        if isinstance(bir_json, str):
            bir_json = bir_json.encode()
        return orig(_split_waits_in_bir(bir_json), tmpdir, neff_name=neff_name)

    wrapped._wait_split_wrapped = True
    bass_utils.compile_bir_kernel = wrapped
    bass2jax.compile_bir_kernel = wrapped


# --------------------------------------------------------------------------
# Patch 2: optional NTFF profiling hook for axon (exec-time measurement).
# Only used when KERNEL_TRACE=1; missing in this image's antenv.
# --------------------------------------------------------------------------
def _install_profile_shim():
    import types, ctypes, contextlib

    if "antenv.axon_hooks" in sys.modules:
        return
    so_path = "/opt/axon/libaxon_pjrt.so"
    if not os.path.exists(so_path):
        return
    lib = ctypes.CDLL(so_path)
    if not hasattr(lib, "axon_start_nrt_profile"):
        return
    lib.axon_start_nrt_profile.argtypes = [ctypes.POINTER(ctypes.c_int64), ctypes.c_size_t]
    lib.axon_start_nrt_profile.restype = ctypes.c_int64
    lib.axon_stop_nrt_profile.argtypes = [ctypes.c_char_p]
    lib.axon_stop_nrt_profile.restype = ctypes.c_int64

    @contextlib.contextmanager
    def _hook(output_dir, device_ids):
        import jax

        jax.devices()
        if device_ids:
            ids = (ctypes.c_int64 * len(device_ids))(*device_ids)
            rc = lib.axon_start_nrt_profile(ids, len(device_ids))
        else:
            rc = lib.axon_start_nrt_profile(None, 0)
        if rc != 0:
            raise RuntimeError(f"axon_start_nrt_profile rc={rc}")
        try:
            yield
        finally:
            n = lib.axon_stop_nrt_profile(str(output_dir).encode())
            print(f"profile: {n} file(s) written to {output_dir}", file=sys.stderr)

    mod = types.ModuleType("antenv.axon_hooks")
    mod.get_axon_ntff_profile_hook = lambda: _hook
    sys.modules["antenv.axon_hooks"] = mod


# --------------------------------------------------------------------------
# Device program (identical for all 8 cores; one batch per core)
# --------------------------------------------------------------------------
def _build_nc():
    import concourse.bass as bass
    import concourse.mybir as mybir
    import concourse.tile as tile

    f32 = mybir.dt.float32
    bf16 = mybir.dt.bfloat16
    Alu = mybir.AluOpType
    Act = mybir.ActivationFunctionType

    nc = bass.Bass("TRN2", debug=False)

    d_maskT = nc.dram_tensor("maskT", [N, N], bf16, kind="ExternalInput")
    d_XT = nc.dram_tensor("XT", [D, N], bf16, kind="ExternalInput")
    d_ZT = nc.dram_tensor("ZT", [DV, N], bf16, kind="ExternalInput")
    d_Wq = nc.dram_tensor("Wq", [D, DK], bf16, kind="ExternalInput")
    d_Wk = nc.dram_tensor("Wk", [D, DK], bf16, kind="ExternalInput")
    d_Wv8 = nc.dram_tensor("Wv8", [DV, DV], bf16, kind="ExternalInput")
    d_out = nc.dram_tensor("out", [N, DV], f32, kind="ExternalOutput")

    HALF = N // 2  # 1024: n-range per pass (phase C of pass 0 overlaps B of pass 1)
    HT = HALF // 128  # 8 n-tiles per half

    with tile.TileContext(nc) as tc:
        with (
            tc.tile_pool(name="prep", bufs=2) as prep,       # XT/ZT staging
            tc.tile_pool(name="wts", bufs=1) as wts,         # Wq/Wk/Wv8/qT2/kT2
            tc.tile_pool(name="maskp", bufs=8) as maskp,     # maskT stream
            tc.tile_pool(name="maskedp", bufs=2 * NT) as maskedp,
            tc.tile_pool(name="zwp", bufs=NT) as zwp,        # bf16 ZW tiles
            tc.tile_pool(name="outp", bufs=3) as outp,       # out staging
            tc.tile_pool(name="rlp", bufs=4) as rlp,         # relu staging (ACT path)
            tc.tile_pool(name="psS", bufs=6, space="PSUM") as psS,   # 6 x 1 bank
            tc.tile_pool(name="psO", bufs=2, space="PSUM") as psO,   # 2 x 1 bank
        ):
            # ---- PE warm-up: dummy matmuls engage the HAM clock un-throttle
            # (K=8/8, 2.4 GHz) while the first DMAs stream in. ----
            wu = wts.tile([128, PW], bf16, tag="wu", name="wu")
            nc.gpsimd.memset(wu[:], 0.0)
            for w in range(12):
                pw = psS.tile([128, PW], f32, tag="psS", name=f"psw{w}")
                nc.tensor.matmul(pw[:], wu[:, :128], wu[:], start=True, stop=True)

            # ---- Phase A: projections + ZW ----
            wq_sb = [wts.tile([128, DK], bf16, tag=f"wq{c}", name=f"wq{c}") for c in range(2)]
            wk_sb = [wts.tile([128, DK], bf16, tag=f"wk{c}", name=f"wk{c}") for c in range(2)]
            for c in range(2):
                nc.gpsimd.dma_start(wq_sb[c][:], d_Wq.ap()[c * 128:(c + 1) * 128, :])
                nc.gpsimd.dma_start(wk_sb[c][:], d_Wk.ap()[c * 128:(c + 1) * 128, :])
            # column-chunked as separate tiles so each qk matmul starts as
            # soon as its own chunk lands
            xt_sb = [[prep.tile([128, PW], bf16, tag=f"xt{c}_{g}", name=f"xt{c}_{g}")
                      for g in range(N // PW)] for c in range(2)]
            for g in range(N // PW):
                for c in range(2):
                    nc.sync.dma_start(
                        xt_sb[c][g][:],
                        d_XT.ap()[c * 128:(c + 1) * 128, g * PW:(g + 1) * PW],
                    )

            # qT2/kT2: 4 column chunks of [128, PW]; rows 0:64 computed, rows
            # 64:128 duplicated so score matmuls can row-pack two m-chunks
            # (tile_position rows 0/64). Chunking lets scores start early.
            qT2 = [wts.tile([128, PW], bf16, tag=f"qT2_{g}", name=f"qT2_{g}") for g in range(N // PW)]
            kT2 = [wts.tile([128, PW], bf16, tag=f"kT2_{g}", name=f"kT2_{g}") for g in range(N // PW)]
            for dsts, w_sb in ((qT2, wq_sb), (kT2, wk_sb)):
                for g in range(N // PW):
                    ps = psS.tile([DK, PW], f32, tag="psS", name="psa_q")
                    for c in range(2):
                        nc.tensor.matmul(
                            ps[:],
                            w_sb[c][:],
                            xt_sb[c][g][:],
                            start=(c == 0),
                            stop=(c == 1),
                        )
                    if g % 2 == 0:
                        nc.vector.tensor_copy(dsts[g][:DK, :], ps[:])
                    else:
                        nc.scalar.activation(dsts[g][:DK, :], ps[:], Act.Copy)
                    nc.scalar.dma_start(dsts[g][DK:2 * DK, :], dsts[g][:DK, :])

            vchunks = [(0, 128), (128, 128), (256, 1)]
            wv_sb = [wts.tile([p, DV], bf16, tag=f"wv{i}", name=f"wv{i}") for i, (v0, p) in enumerate(vchunks)]
            for i, (v0, p) in enumerate(vchunks):
                nc.scalar.dma_start(wv_sb[i][:], d_Wv8.ap()[v0:v0 + p, :])
            zt_sb = [prep.tile([p, N], bf16, tag=f"zt{i}", name=f"zt{i}") for i, (v0, p) in enumerate(vchunks)]
            for i, (v0, p) in enumerate(vchunks):
                nc.scalar.dma_start(zt_sb[i][:], d_ZT.ap()[v0:v0 + p, :])
            zw_sb = []
            for mt in range(NT):
                ps = psS.tile([128, DV], f32, tag="psS", name="psa_zw")
                for i in range(3):
                    nc.tensor.matmul(
                        ps[:],
                        zt_sb[i][:, mt * 128:(mt + 1) * 128],
                        wv_sb[i][:],
                        start=(i == 0),
                        stop=(i == 2),
                    )
                zw = zwp.tile([128, DV], bf16, tag="zw", name=f"zw{mt}")
                nc.scalar.activation(zw[:], ps[:], Act.Copy)
                zw_sb.append(zw)

            # ---- Two passes over n-halves, software-pipelined emission ----
            # B(half) produces masked score tiles; C(half) consumes them.
            # C(half0) groups are emitted interleaved with B(half1) pairs so
            # the scheduler alternates them on the PE and the half-1
            # elementwise stage stays fed.
            masked_sb = {}
            ew = 0  # elementwise work rotation counter

            def emit_b_pair(half, pr):
                nonlocal ew
                n0 = half * HALF
                mts = (2 * pr, 2 * pr + 1)
                mks, mds = [], []
                for j, mt in enumerate(mts):
                    mk = maskp.tile([128, HALF], bf16, tag="mask", name=f"mk{half}_{mt}")
                    nc.sync.dma_start(
                        mk[:], d_maskT.ap()[mt * 128:(mt + 1) * 128, n0:n0 + HALF]
                    )
                    mks.append(mk)
                    md = maskedp.tile([128, HALF], bf16, tag="masked", name=f"md{half}_{mt}")
                    mds.append(md)
                    masked_sb[(half, mt)] = md
                for h in range(SW // PW):
                    pss = []
                    for j, mt in enumerate(mts):
                        ro = DK * j
                        ps = psS.tile([128, PW], f32, tag="psS", name=f"pss{half}_{mt}_{h}")
                        kchunk, kcol = divmod(mt * 128, PW)
                        qchunk = (n0 + h * PW) // PW
                        nc.tensor.matmul(
                            ps[:],
                            kT2[kchunk][ro:ro + DK, kcol:kcol + 128],
                            qT2[qchunk][ro:ro + DK, :],
                            start=True,
                            stop=True,
                        )
                        pss.append(ps)
                    for j, mt in enumerate(mts):
                        sl = slice(h * PW, (h + 1) * PW)
                        if ew % 4 in (0, 2):
                            nc.vector.scalar_tensor_tensor(
                                mds[j][:, sl], pss[j][:], 0.0, mks[j][:, sl],
                                Alu.max, Alu.mult,
                            )
                        else:
                            rl = rlp.tile([128, PW], bf16, tag="rl", name=f"rl{half}_{mt}_{h}")
                            nc.scalar.activation(rl[:], pss[j][:], Act.Relu)
                            eng = nc.gpsimd if ew % 4 == 1 else nc.vector
                            eng.tensor_mul(mds[j][:, sl], rl[:], mks[j][:, sl])
                        ew += 1

            def emit_c_group(half, nt):
                n0 = half * HALF
                ps = psO.tile([128, DV], f32, tag="psO", name=f"pso{half}_{nt}")
                for mt in range(NT):
                    nc.tensor.matmul(
                        ps[:],
                        masked_sb[(half, mt)][:, nt * 128:(nt + 1) * 128],
                        zw_sb[mt][:],
                        start=(mt == 0),
                        stop=(mt == NT - 1),
                    )
                ot = outp.tile([128, DV], f32, tag="out", name=f"ot{half}_{nt}")
                nc.scalar.activation(ot[:], ps[:], Act.Copy)
                nc.sync.dma_start(
                    d_out.ap()[n0 + nt * 128:n0 + (nt + 1) * 128, :], ot[:]
                )

            for pr in range(NT // 2):
                emit_b_pair(0, pr)
            for pr in range(NT // 2):
                emit_b_pair(1, pr)
                emit_c_group(0, pr)
            for nt in range(HT):
                emit_c_group(1, nt)

    return nc


def kernel(Z_l, X_l, M_mask, Wq, Wk, Wv):
    global LAST_EXEC_NS
    _apply_bir_patch()

    trace = os.environ.get("KERNEL_TRACE", "0") == "1"
    if trace:
        _install_profile_shim()

    from concourse.bass_utils import run_bass_kernel_spmd

    Z_l = np.asarray(Z_l, dtype=np.float32)
    X_l = np.asarray(X_l, dtype=np.float32)
    M_mask = np.asarray(M_mask, dtype=np.float32)
    Wq = np.asarray(Wq, dtype=np.float32)
    Wk = np.asarray(Wk, dtype=np.float32)
    Wv = np.asarray(Wv, dtype=np.float32)

    import ml_dtypes
    bf = ml_dtypes.bfloat16

    # Host-side layout prep (transpose + bf16 cast) + scale fold.
    XT = np.ascontiguousarray(X_l.transpose(0, 2, 1)).astype(bf)     # [B, D, N]
    ZT = np.ascontiguousarray(Z_l.transpose(0, 2, 1)).astype(bf)     # [B, DV, N]
    MT = np.ascontiguousarray(M_mask.transpose(0, 2, 1)).astype(bf)  # [B, N(m), N(n)]
    Wv8 = (Wv / np.sqrt(np.float32(DK))).astype(bf)
    Wq = Wq.astype(bf)
    Wk = Wk.astype(bf)

    if "nc" not in _CACHE:
        _CACHE["nc"] = _build_nc()
    nc = _CACHE["nc"]

    in_maps = [
        {
            "maskT": MT[b],
            "XT": XT[b],
            "ZT": ZT[b],
            "Wq": Wq,
            "Wk": Wk,
            "Wv8": Wv8,
        }
        for b in range(B)
    ]
    try:
        res = run_bass_kernel_spmd(nc, in_maps, core_ids=list(range(B)), trace=trace)
    except Exception:
        # A prior (profiled) run can leave an execution unit wedged; the failed
        # attempt clears it and a retry goes through.
        res = run_bass_kernel_spmd(nc, in_maps, core_ids=list(range(B)), trace=trace)
    _CACHE["last_res"] = res
    if trace:
        LAST_EXEC_NS = res.exec_time_ns
    out = np.stack([res.results[b]["out"] for b in range(B)], axis=0)
    return out


# revision 38
# speedup vs baseline: 1.0783x; 1.0783x over previous
"""Trainium2 Bass kernel for nn_AttentionHead_5583457485447 (sparse_attention).

Reference computation (per batch b):
    q = X @ Wq; k = X @ Wk                      # [N, DK]
    s = relu((q @ k.T) / sqrt(DK)) * M_mask     # [N, N]
    out = s @ Z @ Wv                            # [N, DV]

Strategy (8 NeuronCores, data-parallel over batch B=8, one batch per core):
  - Fold 1/sqrt(DK) into Wv (relu is positively homogeneous, rest is linear).
  - Fold Wv into Z on device: ZW = Z @ (Wv/8), so the N x N masked-score
    matrix feeds ONE big matmul: out = masked @ ZW.
  - Compute the score matrix directly in transposed [m, n] layout
    (lhsT = kT, rhs = qT), so it is already in the lhsT layout the second
    matmul needs (contraction over m => m on partitions). No on-chip
    transposes of the N x N matrix.
  - Mask is fed pre-transposed from the host (layout prep only).
  - relu + mask: rotated across engines -- fused DVE scalar_tensor_tensor
    (max(s,0)*maskT), or ACT relu followed by a bf16 DVE/GpSimd multiply.
  - All matmul inputs are bf16 (host-cast); every accumulation is fp32 in
    PSUM. Score matmuls row-pack two K=64 m-chunks into one PE pass via
    tile_position. Two n-half passes let C(half0) overlap B(half1) on the
    PE; a dummy-matmul warm-up engages the HAM full clock early.
"""

import json
import os
import sys

import numpy as np

B, N, D, DK = 8, 2048, 256, 64
DV = D + 1  # 257
NT = N // 128  # 16 tiles along n and along m
PW = 512  # scores matmul moving width
SW = 1024  # fused relu-mask op width (psum tile width, 2 banks)

LAST_EXEC_NS = None
_CACHE = {}


# --------------------------------------------------------------------------
# Patch 1: this container's walrus build rejects instructions carrying more
# than one semaphore wait. Split excess waits onto same-engine NOPs at the
# serialized-BIR level (generic, covers Tile's drains and compute ops).
# --------------------------------------------------------------------------
def _split_waits_in_bir(bir_json: bytes) -> bytes:
    bir = json.loads(bir_json)
    changed = False
    drop_ldw = os.environ.get("KERNEL_DROP_LDW", "0") == "1"
    for fn in bir.get("functions", []):
        for bb in fn.get("blocks", []):
            insts = bb.get("instructions", [])
            if drop_ldw:
                # Remove standalone Ldweights prefetches (the paired Matmult is
                # self-loading: it carries both operands). Merge their sync
                # info into the following Matmult on the same engine.
                merged = []
                pend = {}
                for inst in insts:
                    if inst.get("opcode") == "Ldweights":
                        si = inst.get("sync_info") or {}
                        if si.get("on_wait") or si.get("on_update"):
                            pend.setdefault(inst["engine"], []).append(si)
                        changed = True
                        continue
                    if inst.get("opcode") == "Matmult" and pend.get(inst.get("engine")):
                        tgt = inst.setdefault("sync_info", {"on_update": [], "on_wait": []})
                        tgt.setdefault("on_wait", [])
                        tgt.setdefault("on_update", [])
                        for si in pend.pop(inst["engine"]):
                            tgt["on_wait"] += si.get("on_wait") or []
                            tgt["on_update"] += si.get("on_update") or []
                    merged.append(inst)
                insts = merged
            out = []
            for inst in insts:
                si = inst.get("sync_info")
                ow = (si or {}).get("on_wait") or []
                if len(ow) > 1:
                    changed = True
                    for i, w in enumerate(ow[:-1]):
                        out.append({
                            "debug": inst.get("debug", 0),
                            "engine": inst["engine"],
                            "ins": [],
                            "name": f"{inst['name']}-ws{i}",
                            "opcode": "NoOp",
                            "outs": [],
                            "sync_info": {"on_update": [], "on_wait": [w]},
                            "text_hint": "wait_split",
                        })
                    si["on_wait"] = [ow[-1]]
                out.append(inst)
            bb["instructions"] = out
    return json.dumps(bir).encode() if changed else bir_json


def _apply_bir_patch():
    import concourse.bass_utils as bass_utils
    import concourse.bass2jax as bass2jax

    if os.environ.get("KERNEL_LDW_OPT", "0") == "1":
        rc_orig = bass_utils.run_command
        if not getattr(rc_orig, "_ldw_wrapped", False):
            def rc_wrapped(argv, **kwargs):
                argv = [a.replace("--enable-ldw-opt=false", "--enable-ldw-opt=true")
                        if isinstance(a, str) else a for a in argv]
                return rc_orig(argv, **kwargs)
            rc_wrapped._ldw_wrapped = True
            bass_utils.run_command = rc_wrapped

    orig = bass_utils.compile_bir_kernel
    if getattr(orig, "_wait_split_wrapped", False):
        return

    def wrapped(bir_json, tmpdir, neff_name="file.neff"):
        if isinstance(bir_json, str):
            bir_json = bir_json.encode()
        return orig(_split_waits_in_bir(bir_json), tmpdir, neff_name=neff_name)

    wrapped._wait_split_wrapped = True
    bass_utils.compile_bir_kernel = wrapped
    bass2jax.compile_bir_kernel = wrapped


# --------------------------------------------------------------------------
# Patch 2: optional NTFF profiling hook for axon (exec-time measurement).
# Only used when KERNEL_TRACE=1; missing in this image's antenv.
# --------------------------------------------------------------------------
def _install_profile_shim():
    import types, ctypes, contextlib

    if "antenv.axon_hooks" in sys.modules:
        return
    so_path = "/opt/axon/libaxon_pjrt.so"
    if not os.path.exists(so_path):
        return
    lib = ctypes.CDLL(so_path)
    if not hasattr(lib, "axon_start_nrt_profile"):
        return
    lib.axon_start_nrt_profile.argtypes = [ctypes.POINTER(ctypes.c_int64), ctypes.c_size_t]
    lib.axon_start_nrt_profile.restype = ctypes.c_int64
    lib.axon_stop_nrt_profile.argtypes = [ctypes.c_char_p]
    lib.axon_stop_nrt_profile.restype = ctypes.c_int64

    @contextlib.contextmanager
    def _hook(output_dir, device_ids):
        import jax

        jax.devices()
        if device_ids:
            ids = (ctypes.c_int64 * len(device_ids))(*device_ids)
            rc = lib.axon_start_nrt_profile(ids, len(device_ids))
        else:
            rc = lib.axon_start_nrt_profile(None, 0)
        if rc != 0:
            raise RuntimeError(f"axon_start_nrt_profile rc={rc}")
        try:
            yield
        finally:
            n = lib.axon_stop_nrt_profile(str(output_dir).encode())
            print(f"profile: {n} file(s) written to {output_dir}", file=sys.stderr)

    mod = types.ModuleType("antenv.axon_hooks")
    mod.get_axon_ntff_profile_hook = lambda: _hook
    sys.modules["antenv.axon_hooks"] = mod


# --------------------------------------------------------------------------
# Device program (identical for all 8 cores; one batch per core)
# --------------------------------------------------------------------------
def _build_nc():
    import concourse.bass as bass
    import concourse.mybir as mybir
    import concourse.tile as tile

    f32 = mybir.dt.float32
    bf16 = mybir.dt.bfloat16
    Alu = mybir.AluOpType
    Act = mybir.ActivationFunctionType

    nc = bass.Bass("TRN2", debug=False)

    d_maskT = nc.dram_tensor("maskT", [N, N], bf16, kind="ExternalInput")
    d_XT = nc.dram_tensor("XT", [D, N], bf16, kind="ExternalInput")
    d_ZT = nc.dram_tensor("ZT", [DV, N], bf16, kind="ExternalInput")
    d_Wq = nc.dram_tensor("Wq", [D, DK], bf16, kind="ExternalInput")
    d_Wk = nc.dram_tensor("Wk", [D, DK], bf16, kind="ExternalInput")
    d_Wv8 = nc.dram_tensor("Wv8", [DV, DV], bf16, kind="ExternalInput")
    d_out = nc.dram_tensor("out", [N, DV], f32, kind="ExternalOutput")

    HALF = N // 2  # 1024: n-range per pass (phase C of pass 0 overlaps B of pass 1)
    HT = HALF // 128  # 8 n-tiles per half

    with tile.TileContext(nc) as tc:
        with (
            tc.tile_pool(name="prep", bufs=2) as prep,       # XT/ZT staging
            tc.tile_pool(name="wts", bufs=1) as wts,         # Wq/Wk/Wv8/qT2/kT2
            tc.tile_pool(name="maskp", bufs=8) as maskp,     # maskT stream
            tc.tile_pool(name="maskedp", bufs=2 * NT) as maskedp,
            tc.tile_pool(name="zwp", bufs=NT) as zwp,        # bf16 ZW tiles
            tc.tile_pool(name="outp", bufs=3) as outp,       # out staging
            tc.tile_pool(name="rlp", bufs=4) as rlp,         # relu staging (ACT path)
            tc.tile_pool(name="psS", bufs=6, space="PSUM") as psS,   # 6 x 1 bank
            tc.tile_pool(name="psO", bufs=2, space="PSUM") as psO,   # 2 x 1 bank
        ):
            # ---- PE warm-up: dummy matmuls engage the HAM clock un-throttle
            # (K=8/8, 2.4 GHz) while the first DMAs stream in. ----
            wu = wts.tile([128, PW], bf16, tag="wu", name="wu")
            nc.gpsimd.memset(wu[:], 0.0)
            for w in range(12):
                pw = psS.tile([128, PW], f32, tag="psS", name=f"psw{w}")
                nc.tensor.matmul(pw[:], wu[:, :128], wu[:], start=True, stop=True)

            # ---- Phase A: projections + ZW ----
            wq_sb = [wts.tile([128, DK], bf16, tag=f"wq{c}", name=f"wq{c}") for c in range(2)]
            wk_sb = [wts.tile([128, DK], bf16, tag=f"wk{c}", name=f"wk{c}") for c in range(2)]
            for c in range(2):
                nc.gpsimd.dma_start(wq_sb[c][:], d_Wq.ap()[c * 128:(c + 1) * 128, :])
                nc.gpsimd.dma_start(wk_sb[c][:], d_Wk.ap()[c * 128:(c + 1) * 128, :])
            # column-chunked as separate tiles so each qk matmul starts as
            # soon as its own chunk lands
            xt_sb = [[prep.tile([128, PW], bf16, tag=f"xt{c}_{g}", name=f"xt{c}_{g}")
                      for g in range(N // PW)] for c in range(2)]
            for g in range(N // PW):
                for c in range(2):
                    nc.sync.dma_start(
                        xt_sb[c][g][:],
                        d_XT.ap()[c * 128:(c + 1) * 128, g * PW:(g + 1) * PW],
                    )

            # qT2/kT2: 4 column chunks of [128, PW]; rows 0:64 computed, rows
            # 64:128 duplicated. Alternate score matmuls then use PE row
            # groups 0/64, letting each weight load overlap the in-flight
            # matmul (LDW pull-ahead requires non-conflicting row groups).
            qT2 = [wts.tile([128, PW], bf16, tag=f"qT2_{g}", name=f"qT2_{g}") for g in range(N // PW)]
            kT2 = [wts.tile([128, PW], bf16, tag=f"kT2_{g}", name=f"kT2_{g}") for g in range(N // PW)]
            for dsts, w_sb in ((qT2, wq_sb), (kT2, wk_sb)):
                for g in range(N // PW):
                    ps = psS.tile([DK, PW], f32, tag="psS", name="psa_q")
                    for c in range(2):
                        nc.tensor.matmul(
                            ps[:],
                            w_sb[c][:],
                            xt_sb[c][g][:],
                            start=(c == 0),
                            stop=(c == 1),
                        )
                    if g % 2 == 0:
                        nc.vector.tensor_copy(dsts[g][:DK, :], ps[:])
                    else:
                        nc.scalar.activation(dsts[g][:DK, :], ps[:], Act.Copy)
                    nc.scalar.dma_start(dsts[g][DK:2 * DK, :], dsts[g][:DK, :])

            vchunks = [(0, 128), (128, 128), (256, 1)]
            wv_sb = [wts.tile([p, DV], bf16, tag=f"wv{i}", name=f"wv{i}") for i, (v0, p) in enumerate(vchunks)]
            for i, (v0, p) in enumerate(vchunks):
                nc.scalar.dma_start(wv_sb[i][:], d_Wv8.ap()[v0:v0 + p, :])
            zt_sb = [prep.tile([p, N], bf16, tag=f"zt{i}", name=f"zt{i}") for i, (v0, p) in enumerate(vchunks)]
            for i, (v0, p) in enumerate(vchunks):
                nc.scalar.dma_start(zt_sb[i][:], d_ZT.ap()[v0:v0 + p, :])
            zw_sb = []
            for mt in range(NT):
                ps = psS.tile([128, DV], f32, tag="psS", name="psa_zw")
                for i in range(3):
                    nc.tensor.matmul(
                        ps[:],
                        zt_sb[i][:, mt * 128:(mt + 1) * 128],
                        wv_sb[i][:],
                        start=(i == 0),
                        stop=(i == 2),
                    )
                zw = zwp.tile([128, DV], bf16, tag="zw", name=f"zw{mt}")
                nc.scalar.activation(zw[:], ps[:], Act.Copy)
                zw_sb.append(zw)

            # ---- Two passes over n-halves, software-pipelined emission ----
            # B(half) produces masked score tiles; C(half) consumes them.
            # C(half0) groups are emitted interleaved with B(half1) pairs so
            # the scheduler alternates them on the PE and the half-1
            # elementwise stage stays fed.
            masked_sb = {}
            ew = 0  # elementwise work rotation counter

            def emit_b_pair(half, pr):
                nonlocal ew
                n0 = half * HALF
                mts = (2 * pr, 2 * pr + 1)
                mks, mds = [], []
                for j, mt in enumerate(mts):
                    mk = maskp.tile([128, HALF], bf16, tag="mask", name=f"mk{half}_{mt}")
                    nc.sync.dma_start(
                        mk[:], d_maskT.ap()[mt * 128:(mt + 1) * 128, n0:n0 + HALF]
                    )
                    mks.append(mk)
                    md = maskedp.tile([128, HALF], bf16, tag="masked", name=f"md{half}_{mt}")
                    mds.append(md)
                    masked_sb[(half, mt)] = md
                for h in range(SW // PW):
                    pss = []
                    for j, mt in enumerate(mts):
                        ro = DK * j
                        ps = psS.tile([128, PW], f32, tag="psS", name=f"pss{half}_{mt}_{h}")
                        kchunk, kcol = divmod(mt * 128, PW)
                        qchunk = (n0 + h * PW) // PW
                        nc.tensor.matmul(
                            ps[:],
                            kT2[kchunk][ro:ro + DK, kcol:kcol + 128],
                            qT2[qchunk][ro:ro + DK, :],
                            start=True,
                            stop=True,
                        )
                        pss.append(ps)
                    for j, mt in enumerate(mts):
                        sl = slice(h * PW, (h + 1) * PW)
                        if ew % 4 in (0, 2):
                            nc.vector.scalar_tensor_tensor(
                                mds[j][:, sl], pss[j][:], 0.0, mks[j][:, sl],
                                Alu.max, Alu.mult,
                            )
                        else:
                            rl = rlp.tile([128, PW], bf16, tag="rl", name=f"rl{half}_{mt}_{h}")
                            nc.scalar.activation(rl[:], pss[j][:], Act.Relu)
                            eng = nc.gpsimd if ew % 4 == 1 else nc.vector
                            eng.tensor_mul(mds[j][:, sl], rl[:], mks[j][:, sl])
                        ew += 1

            def emit_c_group(half, nt, mt_range=None, partial_in=None, partial_out=None):
                n0 = half * HALF
                mt_range = mt_range or range(NT)
                ps = psO.tile([128, DV], f32, tag="psO", name=f"pso{half}_{nt}_{mt_range[0]}")
                for i, mt in enumerate(mt_range):
                    nc.tensor.matmul(
                        ps[:],
                        masked_sb[(half, mt)][:, nt * 128:(nt + 1) * 128],
                        zw_sb[mt][:],
                        start=(i == 0),
                        stop=(i == len(mt_range) - 1),
                    )
                if partial_out is not None:
                    nc.scalar.activation(partial_out[:], ps[:], Act.Copy)
                    return
                ot = outp.tile([128, DV], f32, tag="out", name=f"ot{half}_{nt}")
                if partial_in is not None:
                    nc.vector.tensor_add(ot[:], ps[:], partial_in[:])
                else:
                    nc.scalar.activation(ot[:], ps[:], Act.Copy)
                nc.sync.dma_start(
                    d_out.ap()[n0 + nt * 128:n0 + (nt + 1) * 128, :], ot[:]
                )

            for pr in range(NT // 2):
                emit_b_pair(0, pr)
            for pr in range(NT // 2):
                emit_b_pair(1, pr)
                emit_c_group(0, pr)
            for nt in range(HT):
                emit_c_group(1, nt)

    return nc


def kernel(Z_l, X_l, M_mask, Wq, Wk, Wv):
    global LAST_EXEC_NS
    _apply_bir_patch()

    trace = os.environ.get("KERNEL_TRACE", "0") == "1"
    if trace:
        _install_profile_shim()

    from concourse.bass_utils import run_bass_kernel_spmd

    Z_l = np.asarray(Z_l, dtype=np.float32)
    X_l = np.asarray(X_l, dtype=np.float32)
    M_mask = np.asarray(M_mask, dtype=np.float32)
    Wq = np.asarray(Wq, dtype=np.float32)
    Wk = np.asarray(Wk, dtype=np.float32)
    Wv = np.asarray(Wv, dtype=np.float32)

    import ml_dtypes
    bf = ml_dtypes.bfloat16

    # Host-side layout prep (transpose + bf16 cast) + scale fold.
    XT = np.ascontiguousarray(X_l.transpose(0, 2, 1)).astype(bf)     # [B, D, N]
    ZT = np.ascontiguousarray(Z_l.transpose(0, 2, 1)).astype(bf)     # [B, DV, N]
    MT = np.ascontiguousarray(M_mask.transpose(0, 2, 1)).astype(bf)  # [B, N(m), N(n)]
    Wv8 = (Wv / np.sqrt(np.float32(DK))).astype(bf)
    Wq = Wq.astype(bf)
    Wk = Wk.astype(bf)

    if "nc" not in _CACHE:
        _CACHE["nc"] = _build_nc()
    nc = _CACHE["nc"]

    in_maps = [
        {
            "maskT": MT[b],
            "XT": XT[b],
            "ZT": ZT[b],
            "Wq": Wq,
            "Wk": Wk,
            "Wv8": Wv8,
        }
        for b in range(B)
    ]
    try:
        res = run_bass_kernel_spmd(nc, in_maps, core_ids=list(range(B)), trace=trace)
    except Exception:
        # A prior (profiled) run can leave an execution unit wedged; the failed
        # attempt clears it and a retry goes through.
        res = run_bass_kernel_spmd(nc, in_maps, core_ids=list(range(B)), trace=trace)
    _CACHE["last_res"] = res
    if trace:
        LAST_EXEC_NS = res.exec_time_ns
    out = np.stack([res.results[b]["out"] for b in range(B)], axis=0)
    return out
